# revision 4
# baseline (speedup 1.0000x reference)
"""Trainium2 Bass kernel for nn_MimoLinearDynamicalOperator.

Math: y = MIMO IIR filter over time:
  FIR:  v[b,t,o] = sum_{j<NB} sum_i u[b,t-j,i] * b_coeff[j,i,o]
  AR:   x[b,t,o] = sum_{j<NA} a_coeff[j,o] * x[b,t-1-j,o] + v[b,t,o]
The AR part is a *diagonal* (per-output-channel) order-2 recurrence whose
impulse response h_o decays below 1e-30 within ~64 lags (|a|~0.05*N(0,1)),
so it is computed exactly (to fp precision) as a 128-tap causal
convolution.  Initial state x0 is folded in as virtual FIR outputs at
t=-1,-2:  v[-2] = x0[:,0,:],  v[-1] = x0[:,1,:] - a0*x0[:,0,:]  (exact).

Sharding: batch (32 sequences) data-parallel over 8 cores, 4 seq/core.

Device pipeline per core (all matmuls float32r ~ tf32-ish, 1 col/cycle):
  stage 1: v^T = FIR(u): block-diagonal taps [(b,i)=128]x[(b,o)=128],
           time on the free dim, 3 shift-accumulated matmuls per chunk.
  transpose: psum -> f32r sbuf (ACT cast) -> PE transpose -> psum ->
           ACT copy -> v_sb[k = t mod 128, 1+chunk, (b,o)]  (col 0 = x0).
  stage 2: per output channel o: windowed causal-tap matmuls
           (B tile [128,128] on chunk c, A tile [64,64] on chunk c-1),
           psum[r, (b,c)] -> strided copy -> x_sb[r, b, c, o] -> DMA out.
"""

import numpy as np
import ml_dtypes

B, T, CIN, COUT, NB, NA = 32, 16384, 32, 32, 3, 2
NCORES = 8
BL = B // NCORES          # sequences per core
L = 128                   # chunk length (time)
C = T // L                # chunks per sequence
PADT = 128                # zero head columns before t=0 in u input
SLABW = 2048              # stage-1 streaming slab width (timesteps)
NSLAB = T // SLABW
UOVL = 128                # u slab left-overlap (covers the j-shifts)
HALF_C = C // 2           # stage-2 section size in chunks

_CACHE = {}


def _build_program():
    """Build + compile the (SPMD, per-core) Bass program once."""
    if "nc" in _CACHE:
        return _CACHE["nc"]
    from concourse import bacc, bass, mybir, tile

    f32r = mybir.dt.float32r
    f32 = mybir.dt.float32
    MS = bass.MemorySpace

    nc = bacc.Bacc("TRN2", target_bir_lowering=False, debug=False)

    u_in = nc.dram_tensor("u_in", [128, PADT + T], f32r, kind="ExternalInput").ap()
    taps1 = nc.dram_tensor("taps1", [128, NB, 128], f32r, kind="ExternalInput").ap()
    taps2b = nc.dram_tensor("taps2b", [128, COUT, 128], f32r, kind="ExternalInput").ap()
    taps2a = nc.dram_tensor("taps2a", [128, COUT, 128], f32r, kind="ExternalInput").ap()
    initcol = nc.dram_tensor("initcol", [128, 128], f32r, kind="ExternalInput").ap()
    ident_in = nc.dram_tensor("ident_in", [128, 128], f32r, kind="ExternalInput").ap()
    x_out = nc.dram_tensor("x_out", [BL, T, COUT], f32, kind="ExternalOutput").ap()

    # DRAM view of the output matching the sbuf x_half layout:
    # x[b, t, o], t = c*128 + p  ->  [p, b, c, o]
    x_view = x_out.rearrange("b (c p) o -> p b c o", p=L)

    with tile.TileContext(nc) as tc:
        with (
            tc.tile_pool(name="const", bufs=1) as const_pool,
            tc.tile_pool(name="vsb", bufs=1) as vsb_pool,
            tc.tile_pool(name="uslab", bufs=2) as u_pool,
            tc.tile_pool(name="vslab", bufs=2) as v_pool,
            tc.tile_pool(name="xhalf", bufs=2) as x_pool,
            tc.tile_pool(name="ps1", bufs=2, space=MS.PSUM) as ps1_pool,
            tc.tile_pool(name="psT", bufs=2, space=MS.PSUM) as psT_pool,
            tc.tile_pool(name="ps2", bufs=2, space=MS.PSUM) as ps2_pool,
        ):
            taps1_sb = const_pool.tile([128, NB, 128], f32r)
            nc.sync.dma_start(taps1_sb[:], taps1[:])
            taps2b_sb = const_pool.tile([128, COUT, 128], f32r)
            nc.sync.dma_start(taps2b_sb[:], taps2b[:])
            taps2a_sb = const_pool.tile([128, COUT, 128], f32r)
            nc.sync.dma_start(taps2a_sb[:], taps2a[:])
            ident = const_pool.tile([128, 128], f32r)
            nc.sync.dma_start(ident[:], ident_in[:])

            # v_sb[k, cslot, bo]: cslot 0 = virtual chunk -1 (x0 fold-in),
            # cslot 1+c = FIR output chunk c.
            v_sb = vsb_pool.tile([128, 1 + C, 128], f32r)
            nc.sync.dma_start(v_sb[:, 0, :], initcol[:])

            # ---- stage 1 + transpose, slab by slab ----
            for s in range(NSLAB):
                u_slab = u_pool.tile([128, UOVL + SLABW], f32r)
                nc.sync.dma_start(
                    u_slab[:], u_in[:, s * SLABW : s * SLABW + UOVL + SLABW]
                )
                v_slab = v_pool.tile([128, SLABW], f32r)
                for q in range(SLABW // 512):
                    ps1 = ps1_pool.tile([128, 512], f32)
                    base = UOVL + q * 512
                    for j in range(NB):
                        nc.tensor.matmul(
                            ps1[:],
                            taps1_sb[:, j, :],
                            u_slab[:, base - j : base - j + 512],
                            start=(j == 0),
                            stop=(j == NB - 1),
                        )
                    # evacuate + round fp32 -> f32r on the scalar engine
                    nc.scalar.copy(v_slab[:, q * 512 : (q + 1) * 512], ps1[:])
                # PE-transpose the slab, 4 chunks per psum bank
                csl = 1 + s * (SLABW // L)
                for g in range(SLABW // 512):
                    psT = psT_pool.tile([128, 4, 128], f32r)
                    for i in range(4):
                        nc.tensor.transpose(
                            psT[:, i, :],
                            v_slab[:, (4 * g + i) * 128 : (4 * g + i + 1) * 128],
                            ident[:],
                        )
                    nc.scalar.copy(v_sb[:, csl + 4 * g : csl + 4 * g + 4, :], psT[:])

            # ---- stage 2, half-T sections ----
            for half in range(2):
                c0 = half * HALF_C
                x_half = x_pool.tile([128, BL, HALF_C, COUT], f32)
                for o in range(COUT):
                    ps2 = ps2_pool.tile([128, BL * HALF_C], f32)
                    # window B first (start=True covers all 128 rows):
                    # chunk c (cslot c0+1..), full [128,128] taps
                    rhs_b = v_sb[:, c0 + 1 : c0 + 1 + HALF_C, o::32].rearrange(
                        "p c b -> p b c"
                    )
                    nc.tensor.matmul(
                        ps2[:], taps2b_sb[:, o, :], rhs_b, start=True, stop=False
                    )
                    # window A: chunk c-1 (cslot c0..), taps h(128 + r - k)
                    rhs_a = v_sb[:, c0 : c0 + HALF_C, o::32].rearrange(
                        "p c b -> p b c"
                    )
                    nc.tensor.matmul(
                        ps2[:], taps2a_sb[:, o, :], rhs_a, start=False, stop=True
                    )
                    # evacuate psum [r, (b,c)] -> x_half[r, b, c, o]
                    if o % 2 == 0:
                        nc.vector.tensor_copy(x_half[:, :, :, o], ps2[:])
                    else:
                        nc.scalar.copy(x_half[:, :, :, o], ps2[:])
                for bb in range(BL):
                    nc.sync.dma_start(
                        x_view[:, bb, c0 : c0 + HALF_C, :], x_half[:, bb, :, :]
                    )

    nc.compile()
    _CACHE["nc"] = nc
    return nc


def _host_prep(u, x0, a_coeff, b_coeff):
    """Build the 8 per-core input maps (numpy; f32r tensors carry fp32)."""
    u = np.asarray(u, dtype=np.float32)
    x0 = np.asarray(x0, dtype=np.float32)
    a = np.asarray(a_coeff, dtype=np.float64)
    b = np.asarray(b_coeff, dtype=np.float64)

    # impulse response of the per-channel AR recurrence, lags 0..255
    H = np.zeros((256, COUT))
    H[0] = 1.0
    H[1] = a[0]
    for m in range(2, 256):
        H[m] = a[0] * H[m - 1] + a[1] * H[m - 2]
    Hp = np.concatenate([H, np.zeros((1, COUT))], axis=0)

    kk, rr = np.meshgrid(np.arange(L), np.arange(L), indexing="ij")
    lagB = rr - kk
    lagBc = np.where((lagB >= 0) & (lagB < 256), lagB, 256)
    taps2b = Hp[lagBc].transpose(0, 2, 1).astype(np.float32)  # [k, o, r]

    # A tile: tap = h(128 + r - k)  (decays to exact 0 well within range)
    lagA = 128 + rr - kk
    lagAc = np.where((lagA >= 0) & (lagA < 256), lagA, 256)
    taps2a = Hp[lagAc].transpose(0, 2, 1).astype(np.float32)  # [k, o, r]

    taps1 = np.zeros((128, NB, 128), dtype=np.float32)
    for j in range(NB):
        for bb in range(BL):
            taps1[bb * 32 : bb * 32 + 32, j, bb * 32 : bb * 32 + 32] = b[j]

    ident = np.eye(128, dtype=np.float32)

    uc = np.ascontiguousarray(u.transpose(0, 2, 1))  # [B, CIN, T]

    in_maps = []
    for k in range(NCORES):
        arr = np.zeros((128, PADT + T), dtype=np.float32)
        arr[:, PADT:] = uc[k * BL : (k + 1) * BL].reshape(BL * CIN, T)
        ic = np.zeros((128, 128), dtype=np.float64)
        x0k = x0[k * BL : (k + 1) * BL].astype(np.float64)  # [BL, NA, COUT]
        for bb in range(BL):
            ic[126, bb * 32 : bb * 32 + 32] = x0k[bb, 0]
            ic[127, bb * 32 : bb * 32 + 32] = x0k[bb, 1] - a[0] * x0k[bb, 0]
        in_maps.append(
            {
                "u_in": arr,
                "taps1": taps1,
                "taps2b": taps2b,
                "taps2a": taps2a,
                "initcol": ic.astype(np.float32),
                "ident_in": ident,
            }
        )
    return in_maps


def kernel(u, x0, a_coeff, b_coeff):
    from concourse.bass_utils import run_bass_kernel_spmd

    nc = _build_program()
    in_maps = _host_prep(u, x0, a_coeff, b_coeff)
    res = run_bass_kernel_spmd(nc, in_maps, list(range(NCORES)))
    out = np.concatenate([res.results[k]["x_out"] for k in range(NCORES)], axis=0)
    return out.astype(np.float32)
